# revision 71
# baseline (speedup 1.0000x reference)
"""GQA attention with LoRA-Q, tensor-parallel over 8 TRN2 cores.

Sharding (per core i of 8):
  - Q heads 4i..4i+3 (256 q-dims) and KV head i (GQA: repeat_interleave maps
    q heads [4i,4i+4) exactly onto kv head i).
  - Wq (with LoRA folded: Wq_eff = Wq + lora_B @ lora_A), Wk, Wv row-sharded;
    Wo row-parallel on its input (head) dim: each core computes the full-width
    partial y^T from its own 256 head-dims; four per-T-chunk ReduceScatter(add)
    ops (overlapped with attention of later chunks) leave each core its
    256-feature slice of y^T (transposed + concatenated on the host).

Structure is a single fused c-outer pipeline over the four 512-token chunks:
  proj c -> rope c -> attention (head pairs, shared kv) c -> norm c ->
  partial-Wo drain c -> ReduceScatter #c
so the collective and DMA traffic hide under attention of later chunks.

All matmuls in bf16 with fp32 PSUM accumulation; softmax without max
subtraction (scores are bounded: |S/8| <= ~7), denominator fused into the
PV matmul via an appended ones-column on V. RoPE's rotate-half is a signed
permutation matmul on PE (no partition-shuffle DMAs).
"""

import numpy as np
import ml_dtypes

import concourse.bass as bass
import concourse.mybir as mybir
import concourse.tile as tile
from concourse import bacc
from concourse.bass_utils import run_bass_kernel_spmd

BF16 = mybir.dt.bfloat16
F32 = mybir.dt.float32
FP8 = mybir.dt.float8e4

N_CORES = 8
T = 2048
D = 2048
HD = 64          # head dim
NH = 32          # total q heads
NKV = 8          # total kv heads
NH_LOC = NH // N_CORES       # 4 q heads per core
QW = NH_LOC * HD             # 256 q dims per core
P = 128
KT = D // P                  # 16 contraction tiles
CH = 512                     # T-chunk
NCH = T // CH                # 4 chunks
NJ = T // P                  # 16 k-blocks
SCALE = 1.0 / 8.0            # 1/sqrt(64)


def build_bass():
    nc = bacc.Bacc(None, num_devices=N_CORES)

    # I/O
    xT_d = nc.dram_tensor("xT", [D, T], BF16, kind="ExternalInput")
    w_d = nc.dram_tensor("w_all", [D, QW + 2 * HD], BF16, kind="ExternalInput")
    woT_d = nc.dram_tensor("woT", [QW, D], BF16, kind="ExternalInput")
    cos2_d = nc.dram_tensor("cos2", [P, T], BF16, kind="ExternalInput")
    sin2_d = nc.dram_tensor("sin2", [P, T], BF16, kind="ExternalInput")
    mask2_d = nc.dram_tensor("dmask2", [P, 4, 2 * CH], BF16, kind="ExternalInput")
    perm_d = nc.dram_tensor("perm", [P, P], BF16, kind="ExternalInput")
    y_d = nc.dram_tensor("y", [NCH, QW, CH], BF16, kind="ExternalOutput")

    with tile.TileContext(nc, num_cores=N_CORES) as tc:
        _body(nc, tc, xT_d, w_d, woT_d, cos2_d, sin2_d, mask2_d, perm_d, y_d)
    nc.compile()
    return nc


def _body(nc, tc, xT_d, w_d, woT_d, cos2_d, sin2_d, mask2_d, perm_d, y_d):
    import contextlib
    import itertools

    ctx = contextlib.ExitStack()
    with ctx:
        consts = ctx.enter_context(tc.tile_pool(name="consts", bufs=1))
        big = ctx.enter_context(tc.tile_pool(name="big", bufs=1))
        work = ctx.enter_context(tc.tile_pool(name="work", bufs=1))
        rp = ctx.enter_context(tc.tile_pool(name="rp", bufs=2))
        pt_p = ctx.enter_context(tc.tile_pool(name="pt_p", bufs=3))
        nrm = ctx.enter_context(tc.tile_pool(name="nrm", bufs=2))
        pst = ctx.enter_context(tc.tile_pool(name="pst", bufs=2))
        stP = ctx.enter_context(tc.tile_pool(name="stP", bufs=2, space="PSUM"))
        otP = ctx.enter_context(tc.tile_pool(name="otP", bufs=2, space="PSUM"))
        drP = ctx.enter_context(tc.tile_pool(name="drP", bufs=1, space="PSUM"))
        dram = ctx.enter_context(tc.tile_pool(name="dram", bufs=1, space="DRAM"))

        # ---- constant tiles (few big DMAs; the DMA path serializes per
        # dma_start, so count matters more than size)
        w_sb = consts.tile([P, KT, QW + 2 * HD], BF16)
        w_r = w_d.rearrange("(kt p) m -> p kt m", p=P)
        xT_sb = big.tile([P, KT, T], BF16, tag="big", name="xT_sb")
        xT_r = xT_d.rearrange("(kt p) t -> p kt t", p=P)
        nc.sync.dma_start(w_sb[:, 0:8, :], w_r[:, 0:8, :])
        nc.sync.dma_start(xT_sb[:, 0:8, 0:CH], xT_r[:, 0:8, 0:CH])
        nc.sync.dma_start(w_sb[:, 8:KT, :], w_r[:, 8:KT, :])
        nc.sync.dma_start(xT_sb[:, 8:KT, 0:CH], xT_r[:, 8:KT, 0:CH])
        cos2_sb = consts.tile([P, T], BF16)
        nc.sync.dma_start(cos2_sb, cos2_d[:])
        sin2_sb = consts.tile([P, T], BF16)
        nc.sync.dma_start(sin2_sb, sin2_d[:])
        perm_sb = consts.tile([P, P], BF16)
        nc.sync.dma_start(perm_sb, perm_d[:])
        nc.sync.dma_start(xT_sb[:, :, CH : 2 * CH], xT_r[:, :, CH : 2 * CH])
        tri_sb = consts.tile([P, P], BF16)
        nc.sync.dma_start(tri_sb, mask2_d[0:P, 0, 0:P])
        woT2_sb = consts.tile([P, 2, D], BF16)
        woT2_r = woT_d.rearrange("(g p) o -> p g o", p=P)
        nc.sync.dma_start(woT2_sb, woT2_r[:, :, :])
        nc.sync.dma_start(xT_sb[:, :, 2 * CH : 3 * CH], xT_r[:, :, 2 * CH : 3 * CH])
        nc.sync.dma_start(xT_sb[:, :, 3 * CH : 4 * CH], xT_r[:, :, 3 * CH : 4 * CH])

        ones64 = consts.tile([1, HD], BF16)
        nc.vector.memset(ones64, 1.0)

        # v with ones column appended: [tk(P), j, HD+1]
        v_aug = work.tile([P, NJ, HD + 1], BF16)
        nc.vector.memset(v_aug[:, :, HD : HD + 1], 1.0)

        # k^T duplicated on both partition halves so odd heads (whose rope
        # output lives at partitions 64:128) can matmul base-aligned
        kT2 = work.tile([P, T], BF16)
        # rope'd q pairs for all chunks: qAll[64*(h%2)+d, h//2, t]
        qAll = work.tile([P, 2, T], BF16)
        # O^T repacked to 128 partitions: OT128[64*(h%2)+d, h//2, t]
        # (partition p of pair g is local head-dim 128*g+p, matching woT2)
        OT128 = work.tile([P, 2, T], BF16)

        # chunk-major so each chunk's collective sees contiguous DRAM
        partT_dram = dram.tile([NCH, D, CH], BF16)
        partT_r = partT_dram.rearrange("c (ot p) t -> p c ot t", p=P)
        ysT_dram = dram.tile([NCH, QW, CH], BF16)

        def copy_via(idx, dst, src, act_every=2):
            if idx % act_every == 0:
                nc.scalar.copy(dst, src)
            else:
                nc.vector.tensor_copy(dst, src)

        def emit_proj_rope(c):
            """QKV projection + RoPE + t-partitioned V for chunk c, as a
            generator so chunks >= 1 can ride attention's PE bubbles."""
            sl = slice(c * CH, (c + 1) * CH)
            projT = rp.tile([P, 3, CH], BF16, tag="projT")
            for m in (2, 0, 1):
                ps = drP.tile([P, 2 * CH], F32, tag="dr")
                for kt in range(KT):
                    nc.tensor.matmul(
                        ps[:, 0:CH],
                        lhsT=w_sb[:, kt, m * P : (m + 1) * P],
                        rhs=xT_sb[:, kt, sl],
                        start=(kt == 0),
                        stop=(kt == KT - 1),
                    )
                    if kt % 8 == 7:
                        yield
                nc.vector.tensor_copy(projT[:, m, :], ps[:, 0:CH])
                yield
                if m == 2:
                    # RoPE k -> kT2[0:64, sl] + dup to [64:128]
                    ksh = drP.tile([P, 2 * CH], F32, tag="dr")
                    nc.tensor.matmul(
                        ksh[0:HD, 0:CH], lhsT=perm_sb[0:HD, 0:HD],
                        rhs=projT[0:HD, 2, :], start=True, stop=True,
                    )
                    kt2 = rp.tile([P, CH], BF16, tag="t2")
                    nc.vector.tensor_mul(kt2[0:HD, :], ksh[0:HD, 0:CH], sin2_sb[0:HD, sl])
                    veng = nc.vector if c == 0 else nc.gpsimd
                    kt1 = rp.tile([P, CH], BF16, tag="t1")
                    veng.tensor_mul(kt1[0:HD, :], projT[0:HD, 2, :], cos2_sb[0:HD, sl])
                    veng.tensor_add(kT2[0:HD, sl], kt1[0:HD, :], kt2[0:HD, :])
                    nc.gpsimd.dma_start(kT2[HD:P, sl], kT2[0:HD, sl])
                    yield
                    # v in t-partitioned layout (free dim 64 -> 27ns/matmul)
                    for tb in range(4):
                        pv = drP.tile([P, 2 * CH], F32, tag="dr")
                        for kt in range(KT):
                            nc.tensor.matmul(
                                pv[:, 0:HD],
                                lhsT=xT_sb[:, kt, (4 * c + tb) * P : (4 * c + tb + 1) * P],
                                rhs=w_sb[:, kt, QW + HD : QW + 2 * HD],
                                start=(kt == 0),
                                stop=(kt == KT - 1),
                            )
                        nc.vector.tensor_copy(v_aug[:, 4 * c + tb, 0:HD], pv[:, 0:HD])
                        yield
                else:
                    s = m
                    qsh = drP.tile([P, 2 * CH], F32, tag="dr")
                    nc.tensor.matmul(
                        qsh[:, 0:CH], lhsT=perm_sb, rhs=projT[:, s, :],
                        start=True, stop=True,
                    )
                    yield
                    t2 = rp.tile([P, CH], BF16, tag="t2")
                    nc.vector.tensor_mul(t2, qsh[:, 0:CH], sin2_sb[:, sl])
                    veng = nc.vector if c == 0 else nc.gpsimd
                    t1 = rp.tile([P, CH], BF16, tag="t1")
                    veng.tensor_mul(t1, projT[:, s, :], cos2_sb[:, sl])
                    veng.tensor_add(qAll[:, s, sl], t1, t2)
                    yield

        def drain_gen(c, final=False):
            """Chunk c's partial-Wo drain + partT DMAs + ReduceScatter.
            Interleaved into attention (c+1)'s j-loop as PE filler; the final
            drain double-buffers by alternating PSUM between drP and the
            idle stP."""
            sl = slice(c * CH, (c + 1) * CH)
            pstage = pst.tile([P, KT, CH], BF16, tag="pst")
            for ot2 in range(8):
                if final and ot2 % 2 == 1:
                    ps = stP.tile([P, 2, CH], F32, tag="st")
                    ps = ps[:, :, :].rearrange("p a b -> p (a b)")
                else:
                    ps = drP.tile([P, 2 * CH], F32, tag="dr")
                for half in range(2):
                    o_t = 2 * ot2 + half
                    for g in range(2):
                        nc.tensor.matmul(
                            ps[:, half * CH : (half + 1) * CH],
                            lhsT=woT2_sb[:, g, o_t * P : (o_t + 1) * P],
                            rhs=OT128[:, g, sl],
                            start=(g == 0),
                            stop=(g == 1),
                        )
                    yield
                copy_via(ot2 + 1, pstage[:, 2 * ot2 : 2 * ot2 + 2, :],
                         ps, act_every=2 if final else 4)
                if final and ot2 % 2 == 1:
                    hb = 4 * (ot2 // 2)
                    nc.sync.dma_start(
                        partT_r[:, c, hb : hb + 4, :], pstage[:, hb : hb + 4, :]
                    )
                elif not final and ot2 in (3, 7):
                    hb = 8 * (ot2 // 4)
                    nc.sync.dma_start(
                        partT_r[:, c, hb : hb + 8, :], pstage[:, hb : hb + 8, :]
                    )
                yield
            emit_rs(c)

        def emit_rs(c):
            """ReduceScatter chunk c. The y output DMAs are all deferred to
            the end of the program: a y DMA waits ~21us on its collective,
            and on the in-order SP queue that wait would head-block the next
            chunk's partT DMAs, serializing every collective."""
            nc.gpsimd.collective_compute(
                "ReduceScatter",
                mybir.AluOpType.add,
                replica_groups=[list(range(N_CORES))],
                ins=[partT_dram[c, :, :]],
                outs=[ysT_dram[c, :, :]],
            )

        def emit_attention(c, filler):
            """Attention for the 4 local heads of q-chunk c, exp grouped over
            2 adjacent k-blocks; pulls from filler (drain of chunk c-1)
            between ST and PV so PE never idles while Exp runs."""
            nj = 4 * c + 4
            stg = nrm.tile([HD + 1, NH_LOC, CH], BF16, tag="stg")
            sl = slice(c * CH, (c + 1) * CH)
            for h in (0, 2, 1, 3):
                lo = HD * (h % 2)
                qrhs = qAll[lo : lo + HD, h // 2, sl]
                ot = otP.tile([P, CH], F32, tag="ot")

                def do_st(j2):
                    st = stP.tile([P, 2, CH], F32, tag="st")
                    for i in range(2):
                        nc.tensor.matmul(
                            st[:, i, :],
                            lhsT=kT2[lo : lo + HD, (2 * j2 + i) * P : (2 * j2 + i + 1) * P],
                            rhs=qrhs,
                            start=True,
                            stop=True,
                        )
                    return st

                def do_rest(st, j2):
                    pt = pt_p.tile([P, 2, CH], BF16, tag="pt")
                    nc.scalar.activation(
                        pt, st, mybir.ActivationFunctionType.Exp, scale=SCALE
                    )
                    if 2 * j2 >= 4 * c:
                        # diagonal group: per-block column trim. Block at
                        # diag offset d contributes nothing to cols < 128d;
                        # only the 128-wide triangle at [128d, 128d+128)
                        # needs masking; cols >= 128(d+1) are fully valid.
                        for i in range(2):
                            j = 2 * j2 + i
                            d = j - 4 * c
                            nc.vector.tensor_mul(
                                pt[:, i, P * d : P * (d + 1)],
                                pt[:, i, P * d : P * (d + 1)],
                                tri_sb,
                            )
                            nc.tensor.matmul(
                                ot[0 : HD + 1, P * d : CH],
                                lhsT=v_aug[:, j, :],
                                rhs=pt[:, i, P * d : CH],
                                start=(j == 0),
                                stop=(j == nj - 1),
                                skip_group_check=True,
                            )
                    else:
                        for i in range(2):
                            j = 2 * j2 + i
                            nc.tensor.matmul(
                                ot[0 : HD + 1, :],
                                lhsT=v_aug[:, j, :],
                                rhs=pt[:, i, :],
                                start=(j == 0),
                                stop=False,
                                skip_group_check=True,
                            )

                st_cur = do_st(0)
                for j2 in range(nj // 2):
                    st_next = do_st(j2 + 1) if j2 + 1 < nj // 2 else None
                    next(filler, None)
                    do_rest(st_cur, j2)
                    next(filler, None)
                    next(filler, None)
                    st_cur = st_next

                # stage unnormalized O^T + denominator row (bf16)
                copy_via(h, stg[:, h, :], ot[0 : HD + 1, :])
            return stg

        def emit_norm(c, stg):
            """Softmax normalization for chunk c (batched over 4 heads)."""
            sl = slice(c * CH, (c + 1) * CH)
            denT = nrm.tile([1, NH_LOC, CH], BF16, tag="den")
            nc.gpsimd.dma_start(denT[0:1, :, :], stg[HD : HD + 1, :, :])
            rcpT = nrm.tile([1, NH_LOC, CH], BF16, tag="rcp")
            with nc.allow_low_precision("softmax denom in bf16 is fine"):
                nc.vector.reciprocal(rcpT[0:1, 0:2, :], denT[0:1, 0:2, :])
                nc.vector.reciprocal(rcpT[0:1, 2:4, :], denT[0:1, 2:4, :])
            yield
            for h in range(NH_LOC):
                bc = otP.tile([P, CH], F32, tag="ot")
                nc.tensor.matmul(
                    bc[0:HD, :], lhsT=ones64, rhs=rcpT[0:1, h, :],
                    start=True, stop=True,
                )
                if h % 2 == 0:
                    nc.vector.tensor_mul(
                        OT128[0:HD, h // 2, sl], stg[0:HD, h, :], bc[0:HD, :]
                    )
                else:
                    oddt = nrm.tile([HD, CH], BF16, tag="oddt")
                    nc.vector.tensor_mul(oddt, stg[0:HD, h, :], bc[0:HD, :])
                    nc.gpsimd.dma_start(OT128[HD:P, h // 2, sl], oddt)
                yield

        # ---- prologue: chunk 0's proj/rope dense; everything else (proj of
        # later chunks, norm + partial-Wo drain + ReduceScatter of earlier
        # chunks) rides attention's PE bubbles via the filler generators.
        for _ in emit_proj_rope(0):
            pass
        filler = emit_proj_rope(1)
        for c in range(NCH):
            stg = emit_attention(c, filler)
            for _ in filler:
                pass
            gens = []
            if c + 2 < NCH:
                gens.append(emit_proj_rope(c + 2))
            gens.append(emit_norm(c, stg))
            gens.append(drain_gen(c, final=(c == NCH - 1)))
            filler = itertools.chain(*gens)
        # final norm+drain runs dense (its ReduceScatter is emitted inline)
        for _ in filler:
            pass
        y_r = y_d.rearrange("c (r p) t -> p c r t", p=P)
        ys_r = ysT_dram.rearrange("c (r p) t -> p c r t", p=P)
        for c in range(NCH):
            nc.sync.dma_start(y_r[:, c, :, :], ys_r[:, c, :, :])
